# revision 28
# baseline (speedup 1.0000x reference)
"""Trainium2 Bass kernel for nn_ADLS_13022340842024 (moe_routing).

Domain-sharded across 8 NeuronCores: all routing (zeta, alpha) is a
function of domain_id only, so the per-sample LoRA mix collapses to a
per-domain rank-8 weight delta.  Each core handles one domain with the
LoRA delta and tower folded into dense per-domain weights:

    W_eff(d,l) = W_l + zeta[d,l] * SCALING * sum_e alpha[d,l,e] Bm_e A_e

computed exactly in fp32 on host, cast once to bf16.  The device
program is then a pure 3-layer dense MLP + tiny 2-layer tower.  Samples
beyond 2048 per domain (~72 for balanced data) are computed on host in
fp32; cores with fewer samples are zero-padded.

On-device per core:
  * batch as 2 superchunks of 2x512 so each weight tile feeds two
    512-wide matmuls back-to-back (long PE chains, no LoRA epilogues);
  * relu split across Scalar (chunk a) and Vector (chunk b) engines;
  * all DMA issued from Sync/GpSimd queues in strict first-use order so
    compute engines never block behind queue-full waits;
  * tower first-layer chains lag the backbone by one o-tile so the PE
    never waits on an activation;
  * PE pre-warm fills the initial DMA window (HAM un-throttle).
"""
import numpy as np
import ml_dtypes
from contextlib import ExitStack

import concourse.bass as bass
import concourse.tile as tile
from concourse import bacc, mybir
from concourse import bass_utils
from concourse.masks import make_identity

BF16 = ml_dtypes.bfloat16

B, F, V, ED = 16384, 32, 100000, 32
NCORES = 8
BL = B // NCORES                 # 2048 samples per core (one domain)
IN, D0, D1, D2 = 1024, 2048, 1024, 512
D, E, L, R = 8, 8, 3, 4
CH = 512                         # batch chunk per core
NCHUNK = BL // CH                # 4
NSUP = NCHUNK // 2               # 2 superchunks of paired chunks
KT0, KT1, KT2 = IN // 128, D0 // 128, D1 // 128          # 8, 16, 8
OT0, OT1, OT2 = D0 // 128, D1 // 128, D2 // 128          # 16, 8, 4
EPS, EPS_LN, SCALING = 1e-8, 1e-5, 0.25

_CACHED_NC = None


def _build():
    nc = bacc.Bacc("TRN2", target_bir_lowering=False, debug=False)
    f32, f32r, bf16 = (mybir.dt.float32, mybir.dt.float32r, mybir.dt.bfloat16)

    xr_ext = nc.declare_dram_parameter("xr", [128, NCHUNK * KT0 * CH], bf16,
                                       isOutput=False)
    w0_ext = nc.declare_dram_parameter("w0c", [128, OT0 * KT0 * 128], bf16,
                                       isOutput=False)
    w1_ext = nc.declare_dram_parameter("w1c", [128, OT1 * KT1 * 128], bf16,
                                       isOutput=False)
    w2_ext = nc.declare_dram_parameter("w2c", [128, OT2 * KT2 * 128], bf16,
                                       isOutput=False)
    b0_ext = nc.declare_dram_parameter("b0p", [128, OT0], f32, isOutput=False)
    b1_ext = nc.declare_dram_parameter("b1p", [128, OT1], f32, isOutput=False)
    b2_ext = nc.declare_dram_parameter("b2p", [128, OT2], f32, isOutput=False)
    wt_ext = nc.declare_dram_parameter("wtc", [128, OT2 * 8], bf16,
                                       isOutput=False)
    bt1_ext = nc.declare_dram_parameter("bt1c", [40, 1], f32, isOutput=False)
    m2_ext = nc.declare_dram_parameter("m2c", [40, 2], bf16, isOutput=False)
    out_ext = nc.declare_dram_parameter("out", [2, NSUP * CH], f32,
                                        isOutput=True)

    with tile.TileContext(nc) as tc, ExitStack() as ctx:
        wp = ctx.enter_context(tc.tile_pool(name="w", bufs=1))
        hp = ctx.enter_context(tc.tile_pool(name="h", bufs=1))
        sp = ctx.enter_context(tc.tile_pool(name="s", bufs=2))
        pp_mm = ctx.enter_context(tc.tile_pool(name="pmm", bufs=4, space="PSUM"))
        pp_tw = ctx.enter_context(tc.tile_pool(name="ptw", bufs=1, space="PSUM"))
        pp_o2 = ctx.enter_context(tc.tile_pool(name="po2", bufs=1, space="PSUM"))

        # PE pre-warm FIRST in program order (identity build on GpSimd must
        # precede GpSimd's DMA issuance or warmups get scheduled late).
        ident = wp.tile([128, 128], bf16, tag="ident", name="ident")
        make_identity(nc, ident[:, :])
        ps_wu = pp_tw.tile([128, 128], f32, tag="warm")
        for _ in range(48):
            nc.tensor.matmul(out=ps_wu[:, :], lhsT=ident[:, :],
                             rhs=ident[:, :], start=True, stop=True)

        def wtile(ext, total_cols, nsplit, i, name, eng):
            cols = total_cols // nsplit
            t = wp.tile([128, cols], bf16, tag=f"{name}{i}", name=f"{name}{i}")
            eng.dma_start(out=t[:, :], in_=ext[:, i * cols:(i + 1) * cols])
            return t

        def aux(ext, shape, dt, name, eng):
            t = wp.tile(shape, dt, tag=name, name=name)
            eng.dma_start(out=t[:, :], in_=ext[:, :])
            return t

        XC = KT0 * CH
        W0C, W1C, W2C = OT0 * KT0 * 128, OT1 * KT1 * 128, OT2 * KT2 * 128
        OB0, OB1, OB2 = OT0 // 8, OT1 // 4, OT2 // 4  # o-blocks per w tile

        xt = [None] * NCHUNK
        w0c, w1c, w2c = [None] * 8, [None] * 4, [None] * 4

        # DMA issue order = first-use order on two queues (Sync=qA,
        # GpSimd=qB).  Chunk-0's x arrives as four k-pair subtiles and
        # w0's first four o-blocks as single-block tiles, so the first
        # chain starts ~4us after DMA go and streams behind the DMAs.
        KH = 2 * CH
        x0s = [wp.tile([128, KH], bf16, tag=f"x0s{t}", name=f"x0s{t}")
               for t in range(4)]
        w0b = [wp.tile([128, KT0 * 128], bf16, tag=f"w0b{o}", name=f"w0b{o}")
               for o in range(4)]
        xt[1] = wp.tile([128, XC], bf16, tag="x1", name="x1")
        # prologue on FOUR queues: vector/scalar each issue one DMA
        # before any compute lands on them, so the first L0 chain has
        # weights+x ~2us after DMA go.
        # qD (scalar): three early slots only, long before any ACTIVATE
        nc.scalar.dma_start(out=x0s[0][:, :], in_=xr_ext[:, 0:KH])
        nc.scalar.dma_start(out=w0b[1][:, :],
                            in_=w0_ext[:, KT0 * 128:2 * KT0 * 128])
        nc.scalar.dma_start(out=w0b[3][:, :],
                            in_=w0_ext[:, 3 * KT0 * 128:4 * KT0 * 128])
        # qA (sync)
        nc.sync.dma_start(out=x0s[2][:, :], in_=xr_ext[:, 2 * KH:3 * KH])
        nc.sync.dma_start(out=w0b[0][:, :], in_=w0_ext[:, 0:KT0 * 128])
        w0c[2] = wtile(w0_ext, W0C, 8, 2, "w0", nc.sync)
        nc.sync.dma_start(out=xt[1][:, 0:XC // 2], in_=xr_ext[:, XC:XC + XC // 2])
        w0c[4] = wtile(w0_ext, W0C, 8, 4, "w0", nc.sync)
        w0c[6] = wtile(w0_ext, W0C, 8, 6, "w0", nc.sync)
        w1c[0] = wtile(w1_ext, W1C, 4, 0, "w1", nc.sync)
        w1c[2] = wtile(w1_ext, W1C, 4, 2, "w1", nc.sync)
        w2c[0] = wtile(w2_ext, W2C, 4, 0, "w2", nc.sync)
        w2c[2] = wtile(w2_ext, W2C, 4, 2, "w2", nc.sync)
        wtc = wtile(wt_ext, OT2 * 8, 1, 0, "wt", nc.sync)
        t = wp.tile([128, XC], bf16, tag="x2", name="x2")
        nc.sync.dma_start(out=t[:, :], in_=xr_ext[:, 2 * XC:3 * XC])
        xt[2] = t
        # qB (gpsimd) -- critical tiles first, aux strictly after
        nc.gpsimd.dma_start(out=x0s[1][:, :], in_=xr_ext[:, KH:2 * KH])
        nc.gpsimd.dma_start(out=x0s[3][:, :], in_=xr_ext[:, 3 * KH:4 * KH])
        nc.gpsimd.dma_start(out=w0b[2][:, :],
                            in_=w0_ext[:, 2 * KT0 * 128:3 * KT0 * 128])
        b0p = aux(b0_ext, [128, OT0], f32, "b0p", nc.gpsimd)
        nc.gpsimd.dma_start(out=xt[1][:, XC // 2:XC],
                            in_=xr_ext[:, XC + XC // 2:2 * XC])
        w0c[3] = wtile(w0_ext, W0C, 8, 3, "w0", nc.gpsimd)
        w0c[5] = wtile(w0_ext, W0C, 8, 5, "w0", nc.gpsimd)
        w0c[7] = wtile(w0_ext, W0C, 8, 7, "w0", nc.gpsimd)
        b1p = aux(b1_ext, [128, OT1], f32, "b1p", nc.gpsimd)
        w1c[1] = wtile(w1_ext, W1C, 4, 1, "w1", nc.gpsimd)
        w1c[3] = wtile(w1_ext, W1C, 4, 3, "w1", nc.gpsimd)
        b2p = aux(b2_ext, [128, OT2], f32, "b2p", nc.gpsimd)
        w2c[1] = wtile(w2_ext, W2C, 4, 1, "w2", nc.gpsimd)
        w2c[3] = wtile(w2_ext, W2C, 4, 3, "w2", nc.gpsimd)
        bt1c = aux(bt1_ext, [40, 1], f32, "bt1c", nc.gpsimd)
        m2c = aux(m2_ext, [40, 2], bf16, "m2c", nc.gpsimd)
        t = wp.tile([128, XC], bf16, tag="x3", name="x3")
        nc.gpsimd.dma_start(out=t[:, :], in_=xr_ext[:, 3 * XC:4 * XC])
        xt[3] = t

        relu = mybir.ActivationFunctionType.Relu

        def w0sl(o, k):
            if o < 4:
                return w0b[o][:, k * 128:(k + 1) * 128]
            return w0c[o // OB0][:, (o % OB0) * KT0 * 128 + k * 128:
                                 (o % OB0) * KT0 * 128 + (k + 1) * 128]

        def w1sl(o, k):
            return w1c[o // OB1][:, (o % OB1) * KT1 * 128 + k * 128:
                                 (o % OB1) * KT1 * 128 + (k + 1) * 128]

        def w2sl(o, k):
            return w2c[o // OB2][:, (o % OB2) * KT2 * 128 + k * 128:
                                 (o % OB2) * KT2 * 128 + (k + 1) * 128]

        def layer(rhs0, rhs1, kt, ot, wsl, bias_tile, out0, out1,
                  tw=None, o_start=0):
            """One dense layer on the superchunk's chunk pair.

            rhs0/rhs1: fn(k) -> [128, CH] AP.  Each weight tile feeds two
            matmuls.  tw=[16, CH] psum interleaves the tower first-layer
            chains lagged one o-tile behind the backbone.
            """
            for o in range(o_start, ot):
                ps0 = pp_mm.tile([128, CH], f32, tag="mm")
                ps1 = pp_mm.tile([128, CH], f32, tag="mm")
                for k in range(kt):
                    lhsT = wsl(o, k)
                    nc.tensor.matmul(out=ps0[:, :], lhsT=lhsT, rhs=rhs0(k),
                                     start=(k == 0), stop=(k == kt - 1))
                    nc.tensor.matmul(out=ps1[:, :], lhsT=lhsT, rhs=rhs1(k),
                                     start=(k == 0), stop=(k == kt - 1))
                if tw is not None and o > 0:
                    nc.tensor.matmul(out=tw[0:8, :],
                                     lhsT=wtc[:, (o - 1) * 8:o * 8],
                                     rhs=out0[o - 1][:, :],
                                     start=(o == 1), stop=False,
                                     tile_position=(0, 0))
                    nc.tensor.matmul(out=tw[32:40, :],
                                     lhsT=wtc[:, (o - 1) * 8:o * 8],
                                     rhs=out1[o - 1][:, :],
                                     start=(o == 1), stop=False,
                                     tile_position=(0, 32))
                nc.scalar.activation(out=out0[o][:, :], in_=ps0[:, :],
                                     func=relu, bias=bias_tile[:, o:o + 1],
                                     scale=1.0)
                nc.vector.tensor_scalar(out=out1[o][:, :], in0=ps1[:, :],
                                        scalar1=bias_tile[:, o:o + 1],
                                        scalar2=0.0,
                                        op0=mybir.AluOpType.add,
                                        op1=mybir.AluOpType.max)
            if tw is not None:
                o = ot - 1
                nc.tensor.matmul(out=tw[0:8, :], lhsT=wtc[:, o * 8:(o + 1) * 8],
                                 rhs=out0[o][:, :], start=False, stop=True,
                                 tile_position=(0, 0))
                nc.tensor.matmul(out=tw[32:40, :],
                                 lhsT=wtc[:, o * 8:(o + 1) * 8],
                                 rhs=out1[o][:, :], start=False, stop=True,
                                 tile_position=(0, 32))

        def tower_tail(s, ps_tw):
            t1s = sp.tile([40, CH], bf16, tag="t1s")
            nc.vector.tensor_scalar(out=t1s[:, :], in0=ps_tw[:, :],
                                    scalar1=bt1c[:, :], scalar2=0.0,
                                    op0=mybir.AluOpType.add,
                                    op1=mybir.AluOpType.max)
            ps_l = pp_o2.tile([2, CH], f32, tag="twl")
            nc.tensor.matmul(out=ps_l[:, :], lhsT=m2c[:, :], rhs=t1s[:, :],
                             start=True, stop=True)
            outc = sp.tile([2, CH], f32, tag="oc")
            nc.vector.tensor_copy(out=outc[:, :], in_=ps_l[:, :])
            nc.sync.dma_start(out=out_ext[:, s * CH:(s + 1) * CH],
                              in_=outc[:, :])

        for s in range(NSUP):
            c0 = 2 * s
            if s == 0:
                rx0 = lambda k: x0s[k // 2][:, (k % 2) * CH:(k % 2 + 1) * CH]
            else:
                rx0 = lambda k, _t=xt[c0]: _t[:, k * CH:(k + 1) * CH]
            rx1 = lambda k, _t=xt[c0 + 1]: _t[:, k * CH:(k + 1) * CH]
            h1a = [hp.tile([128, CH], bf16, name=f"h1a_{o}", tag=f"h1a_{o}")
                   for o in range(OT0)]
            h1b = [hp.tile([128, CH], bf16, name=f"h1b_{o}", tag=f"h1b_{o}")
                   for o in range(OT0)]
            if s == 0:
                # de-paired o0..o5: chunk-a chains run on x0 alone while x1
                # is still in flight, then chunk-b catches up
                for o in range(6):
                    ps = pp_mm.tile([128, CH], f32, tag="mm")
                    for k in range(KT0):
                        nc.tensor.matmul(out=ps[:, :], lhsT=w0sl(o, k),
                                         rhs=rx0(k), start=(k == 0),
                                         stop=(k == KT0 - 1))
                    nc.scalar.activation(out=h1a[o][:, :], in_=ps[:, :],
                                         func=relu, bias=b0p[:, o:o + 1],
                                         scale=1.0)
                for o in range(6):
                    ps = pp_mm.tile([128, CH], f32, tag="mm")
                    for k in range(KT0):
                        nc.tensor.matmul(out=ps[:, :], lhsT=w0sl(o, k),
                                         rhs=rx1(k), start=(k == 0),
                                         stop=(k == KT0 - 1))
                    nc.vector.tensor_scalar(out=h1b[o][:, :], in0=ps[:, :],
                                            scalar1=b0p[:, o:o + 1],
                                            scalar2=0.0,
                                            op0=mybir.AluOpType.add,
                                            op1=mybir.AluOpType.max)
                layer(rx0, rx1, KT0, OT0, w0sl, b0p, h1a, h1b, o_start=6)
            else:
                layer(rx0, rx1, KT0, OT0, w0sl, b0p, h1a, h1b)
            rh1a = lambda k: h1a[k][:, :]
            rh1b = lambda k: h1b[k][:, :]
            h2a = [hp.tile([128, CH], bf16, name=f"h2a_{o}", tag=f"h2a_{o}")
                   for o in range(OT1)]
            h2b = [hp.tile([128, CH], bf16, name=f"h2b_{o}", tag=f"h2b_{o}")
                   for o in range(OT1)]
            layer(rh1a, rh1b, KT1, OT1, w1sl, b1p, h2a, h2b)
            rh2a = lambda k: h2a[k][:, :]
            rh2b = lambda k: h2b[k][:, :]
            h3a = [hp.tile([128, CH], bf16, name=f"h3a_{o}", tag=f"h3a_{o}")
                   for o in range(OT2)]
            h3b = [hp.tile([128, CH], bf16, name=f"h3b_{o}", tag=f"h3b_{o}")
                   for o in range(OT2)]
            ps_tw = pp_tw.tile([40, CH], f32, tag="tw")
            layer(rh2a, rh2b, KT2, OT2, w2sl, b2p, h3a, h3b, tw=ps_tw)
            tower_tail(s, ps_tw)

    nc.compile()
    return nc


def get_nc():
    global _CACHED_NC
    if _CACHED_NC is None:
        _CACHED_NC = _build()
    return _CACHED_NC


# ---------------- host-side math (exact fp32 mirror of the reference) -------

def _softplus(x):
    return np.logaddexp(0.0, x)


def _ln(x, g, b):
    m = x.mean(-1, keepdims=True)
    v = ((x - m) ** 2).mean(-1, keepdims=True)
    return g * (x - m) / np.sqrt(v + EPS_LN) + b


def _softmax(x):
    e = np.exp(x - x.max(-1, keepdims=True))
    return e / e.sum(-1, keepdims=True)


def _topk_sparse(p, k):
    idx = np.argsort(-p, axis=-1, kind="stable")[..., :k]
    mask = np.zeros_like(p)
    np.put_along_axis(mask, idx, 1.0, axis=-1)
    s = p * mask
    return s / np.maximum(s.sum(-1, keepdims=True), EPS)


def _routing_tables(dom_emb, layer_pos, gate_logits, Wi1, bi1, gi, bti, Wi2,
                    bi2, Wr1, br1, gr, btr, Wr2, br2):
    gate = _softplus(gate_logits.astype(np.float32))
    Rg = np.eye(D, dtype=np.float32) * gate
    Rg = Rg / np.maximum(Rg.sum(1, keepdims=True), EPS)
    hd = Rg @ dom_emb.astype(np.float32)                      # [D, 64]
    ri = np.concatenate([
        np.broadcast_to(hd[:, None, :], (D, L, hd.shape[-1])),
        np.broadcast_to(layer_pos[None].astype(np.float32), (D, L, layer_pos.shape[-1])),
    ], axis=-1)                                               # [D, L, 96]
    hi = np.maximum(_ln(ri @ Wi1.T + bi1, gi, bti), 0.0)
    scores = (hi @ Wi2.T + bi2)[..., 0]
    scores = scores - scores.max(-1, keepdims=True)
    phi = _softmax(scores)
    zeta = _topk_sparse(phi, min(2, L))                       # [D, L]
    hr = np.maximum(_ln(ri @ Wr1.T + br1, gr, btr), 0.0)
    alpha = _topk_sparse(_softmax(hr @ Wr2.T + br2), 2)       # [D, L, E]
    return zeta.astype(np.float32), alpha.astype(np.float32)


def _pack_w(W, kt, ot):
    """[out, in] -> [128, ot*kt*128] with (p, o, k, cc) layout (lhsT blocks)."""
    Wr = np.asarray(W, np.float32).reshape(ot, 128, kt, 128)   # [o, cc, k, p]
    return np.ascontiguousarray(
        Wr.transpose(3, 0, 2, 1).reshape(128, ot * kt * 128)).astype(BF16)


def kernel(field_idx, domain_id, emb_table, W0, b0, W1, b1, W2, b2,
           A0, Bm0, A1, Bm1, A2, Bm2, dom_emb, layer_pos, gate_logits,
           Wi1, bi1, gi, bti, Wi2, bi2, Wr1, br1, gr, btr, Wr2, br2,
           Wt1, bt1, Wt2, bt2):
    field_idx = np.asarray(field_idx)
    domain_id = np.asarray(domain_id).astype(np.int64)
    emb = np.asarray(emb_table, dtype=np.float32)
    W_list = [np.asarray(W, np.float32) for W in (W0, W1, W2)]
    b_list = [np.asarray(b, np.float32) for b in (b0, b1, b2)]
    A_list = [np.asarray(A, np.float32) for A in (A0, A1, A2)]
    Bm_list = [np.asarray(Bm, np.float32) for Bm in (Bm0, Bm1, Bm2)]
    Wt1 = np.asarray(Wt1, np.float32)
    bt1 = np.asarray(bt1, np.float32)
    Wt2 = np.asarray(Wt2, np.float32)
    bt2 = np.asarray(bt2, np.float32)

    zeta, alpha = _routing_tables(
        np.asarray(dom_emb), np.asarray(layer_pos), np.asarray(gate_logits),
        np.asarray(Wi1), np.asarray(bi1), np.asarray(gi), np.asarray(bti),
        np.asarray(Wi2), np.asarray(bi2), np.asarray(Wr1), np.asarray(br1),
        np.asarray(gr), np.asarray(btr), np.asarray(Wr2), np.asarray(br2))

    # fold LoRA delta into per-domain dense weights (exact fp32)
    kts = [KT0, KT1, KT2]
    ots = [OT0, OT1, OT2]
    deltas = []     # deltas[l][d] = zeta*scale * sum_e alpha Bm_e A_e
    for l in range(3):
        Aa, Bb = A_list[l], Bm_list[l]
        dl = []
        for d in range(D):
            w = alpha[d, l, :] * zeta[d, l] * SCALING          # [E]
            act = np.nonzero(w)[0]
            acc = np.zeros((Bb.shape[1], Aa.shape[2]), np.float32)
            for e in act:
                acc += w[e] * (Bb[e] @ Aa[e])
            dl.append(acc)
        deltas.append(dl)

    shared = {
        "b0p": np.ascontiguousarray(b_list[0].reshape(OT0, 128).T),
        "b1p": np.ascontiguousarray(b_list[1].reshape(OT1, 128).T),
        "b2p": np.ascontiguousarray(b_list[2].reshape(OT2, 128).T),
    }

    # host embedding lookup -> feature-major bf16 [IN, B]
    x = emb[field_idx.astype(np.int64)].reshape(B, IN)
    xT = x.T.astype(BF16)

    # assign each core one domain; first BL samples of that domain on
    # device, remainder ("spill") on host in fp32
    order = []            # per core: sample indices (len <= BL)
    spill_idx = []
    for d in range(D):
        idx = np.nonzero(domain_id == d)[0]
        order.append(idx[:BL])
        if idx.shape[0] > BL:
            spill_idx.append(idx[BL:])
    spill_idx = (np.concatenate(spill_idx) if spill_idx
                 else np.empty((0,), np.int64))

    in_maps = []
    for d in range(NCORES):
        idx = order[d]
        xc = np.zeros((IN, BL), dtype=BF16)
        xc[:, :idx.shape[0]] = xT[:, idx]
        xr = np.ascontiguousarray(
            xc.reshape(KT0, 128, NCHUNK, CH).transpose(1, 2, 0, 3)
            .reshape(128, NCHUNK * KT0 * CH))
        m = dict(shared)
        m["xr"] = xr
        for l, nm in enumerate(("w0c", "w1c", "w2c")):
            m[nm] = _pack_w(W_list[l] + deltas[l][d], kts[l], ots[l])
        # tower: Wt1[d]: [8, 512] -> lhsT [512, 8] as (p, k, cc)
        wtt = Wt1[d].T.reshape(OT2, 128, 8)                    # [k, p, cc]
        m["wtc"] = np.ascontiguousarray(
            wtt.transpose(1, 0, 2).reshape(128, OT2 * 8)).astype(BF16)
        btc = np.zeros((40, 1), np.float32)
        btc[0:8, 0] = bt1[d]
        btc[32:40, 0] = bt1[d]
        m["bt1c"] = btc
        m2 = np.zeros((40, 2), np.float32)
        m2[0:8, 0] = Wt2[d, 0, :]
        m2[32:40, 1] = Wt2[d, 0, :]
        m["m2c"] = m2.astype(BF16)
        in_maps.append(m)

    nc = get_nc()
    res = bass_utils.run_bass_kernel_spmd(nc, in_maps, core_ids=list(range(NCORES)))
    out = np.empty(B, dtype=np.float32)
    for d in range(NCORES):
        lg = np.asarray(res.results[d]["out"], np.float32)     # [2, NSUP*CH]
        idx = order[d]
        j = np.arange(idx.shape[0])
        s, r = j // (2 * CH), j % (2 * CH)
        half, jj = r // CH, r % CH
        out[idx] = lg[half, s * CH + jj] + bt2[d, 0]

    # spill samples: exact fp32 on host
    if spill_idx.shape[0]:
        hs = x[spill_idx]                                      # [n, IN]
        ds = domain_id[spill_idx]
        for l in range(3):
            base = hs @ W_list[l].T + b_list[l]
            dlt = np.stack([deltas[l][d] for d in range(D)], 0)  # [D, out, in]
            lora = np.einsum('ni,noi->no', hs, dlt[ds])
            hs = np.maximum(base + lora, 0.0)
        t1 = np.maximum(np.einsum('nf,nof->no', hs, Wt1[ds]) + bt1[ds], 0.0)
        out[spill_idx] = (np.einsum('no,nio->ni', t1, Wt2[ds])[:, 0]
                          + bt2[ds, 0])
    return out


# revision 29
# speedup vs baseline: 1.1952x; 1.1952x over previous
"""Trainium2 Bass kernel for nn_ADLS_13022340842024 (moe_routing).

Domain-sharded across 8 NeuronCores: all routing (zeta, alpha) is a
function of domain_id only, so the per-sample LoRA mix collapses to a
per-domain rank-8 weight delta.  Each core handles one domain with the
LoRA delta and tower folded into dense per-domain weights:

    W_eff(d,l) = W_l + zeta[d,l] * SCALING * sum_e alpha[d,l,e] Bm_e A_e

computed exactly in fp32 on host, cast once to bf16.  The device
program is then a pure 3-layer dense MLP + tiny 2-layer tower.  Samples
beyond 2048 per domain (~72 for balanced data) are computed on host in
fp32; cores with fewer samples are zero-padded.

On-device per core:
  * batch as 2 superchunks of 2x512 so each weight tile feeds two
    512-wide matmuls back-to-back (long PE chains, no LoRA epilogues);
  * relu split across Scalar (chunk a) and Vector (chunk b) engines;
  * DMA issued from Sync/GpSimd/Scalar queues in strict criticality
    order (prologue tiles literally first per queue -- DMA descriptor
    issue blocks invisibly while earlier ring entries are in flight);
  * tower first-layer chains lag the backbone by one o-tile so the PE
    never waits on an activation;
  * PE pre-warm fills the initial DMA window (HAM un-throttle).
"""
import numpy as np
import ml_dtypes
from contextlib import ExitStack

import concourse.bass as bass
import concourse.tile as tile
from concourse import bacc, mybir
from concourse import bass_utils
from concourse.masks import make_identity

BF16 = ml_dtypes.bfloat16

B, F, V, ED = 16384, 32, 100000, 32
NCORES = 8
BL = B // NCORES                 # 2048 samples per core (one domain)
IN, D0, D1, D2 = 1024, 2048, 1024, 512
D, E, L, R = 8, 8, 3, 4
CH = 512                         # batch chunk per core
NCHUNK = BL // CH                # 4
NSUP = NCHUNK // 2               # 2 superchunks of paired chunks
KT0, KT1, KT2 = IN // 128, D0 // 128, D1 // 128          # 8, 16, 8
OT0, OT1, OT2 = D0 // 128, D1 // 128, D2 // 128          # 16, 8, 4
EPS, EPS_LN, SCALING = 1e-8, 1e-5, 0.25

_CACHED_NC = None


def _build():
    nc = bacc.Bacc("TRN2", target_bir_lowering=False, debug=False)
    f32, f32r, bf16 = (mybir.dt.float32, mybir.dt.float32r, mybir.dt.bfloat16)

    xr_ext = nc.declare_dram_parameter("xr", [128, NCHUNK * KT0 * CH], bf16,
                                       isOutput=False)
    w0_ext = nc.declare_dram_parameter("w0c", [128, OT0 * KT0 * 128], bf16,
                                       isOutput=False)
    w1_ext = nc.declare_dram_parameter("w1c", [128, OT1 * KT1 * 128], bf16,
                                       isOutput=False)
    w2_ext = nc.declare_dram_parameter("w2c", [128, OT2 * KT2 * 128], bf16,
                                       isOutput=False)
    b0_ext = nc.declare_dram_parameter("b0p", [128, OT0], f32, isOutput=False)
    b1_ext = nc.declare_dram_parameter("b1p", [128, OT1], f32, isOutput=False)
    b2_ext = nc.declare_dram_parameter("b2p", [128, OT2], f32, isOutput=False)
    wt_ext = nc.declare_dram_parameter("wtc", [128, OT2 * 8], bf16,
                                       isOutput=False)
    bt1_ext = nc.declare_dram_parameter("bt1c", [40, 1], f32, isOutput=False)
    m2_ext = nc.declare_dram_parameter("m2c", [40, 2], bf16, isOutput=False)
    out_ext = nc.declare_dram_parameter("out", [2, NSUP * CH], f32,
                                        isOutput=True)

    with tile.TileContext(nc) as tc, ExitStack() as ctx:
        wp = ctx.enter_context(tc.tile_pool(name="w", bufs=1))
        hp = ctx.enter_context(tc.tile_pool(name="h", bufs=1))
        sp = ctx.enter_context(tc.tile_pool(name="s", bufs=2))
        pp_mm = ctx.enter_context(tc.tile_pool(name="pmm", bufs=4, space="PSUM"))
        pp_tw = ctx.enter_context(tc.tile_pool(name="ptw", bufs=1, space="PSUM"))
        pp_o2 = ctx.enter_context(tc.tile_pool(name="po2", bufs=1, space="PSUM"))

        # PE pre-warm FIRST in program order (identity build on GpSimd must
        # precede GpSimd's DMA issuance or warmups get scheduled late).
        ident = wp.tile([128, 128], bf16, tag="ident", name="ident")
        make_identity(nc, ident[:, :])
        ps_wu = pp_tw.tile([128, 128], f32, tag="warm")
        for _ in range(48):
            nc.tensor.matmul(out=ps_wu[:, :], lhsT=ident[:, :],
                             rhs=ident[:, :], start=True, stop=True)

        def wtile(ext, total_cols, nsplit, i, name, eng):
            cols = total_cols // nsplit
            t = wp.tile([128, cols], bf16, tag=f"{name}{i}", name=f"{name}{i}")
            eng.dma_start(out=t[:, :], in_=ext[:, i * cols:(i + 1) * cols])
            return t

        def aux(ext, shape, dt, name, eng):
            t = wp.tile(shape, dt, tag=name, name=name)
            eng.dma_start(out=t[:, :], in_=ext[:, :])
            return t

        XC = KT0 * CH
        W0C, W1C, W2C = OT0 * KT0 * 128, OT1 * KT1 * 128, OT2 * KT2 * 128
        OB0, OB1, OB2 = OT0 // 8, OT1 // 4, OT2 // 4  # o-blocks per w tile

        xt = [None] * NCHUNK
        w0c, w1c, w2c = [None] * 8, [None] * 4, [None] * 4

        # DMA issue order = first-use order on two queues (Sync=qA,
        # GpSimd=qB).  Chunk-0's x arrives as four k-pair subtiles and
        # w0's first four o-blocks as single-block tiles, so the first
        # chain starts ~4us after DMA go and streams behind the DMAs.
        KH = 2 * CH
        x0s = [wp.tile([128, KH], bf16, tag=f"x0s{t}", name=f"x0s{t}")
               for t in range(4)]
        w0b = [wp.tile([128, KT0 * 128], bf16, tag=f"w0b{o}", name=f"w0b{o}")
               for o in range(4)]
        xt[1] = wp.tile([128, XC], bf16, tag="x1", name="x1")
        # prologue on FOUR queues: vector/scalar each issue one DMA
        # before any compute lands on them, so the first L0 chain has
        # weights+x ~2us after DMA go.
        # qD (scalar): three early slots only, long before any ACTIVATE
        nc.scalar.dma_start(out=x0s[0][:, :], in_=xr_ext[:, 0:KH])
        nc.scalar.dma_start(out=w0b[1][:, :],
                            in_=w0_ext[:, KT0 * 128:2 * KT0 * 128])
        nc.scalar.dma_start(out=w0b[3][:, :],
                            in_=w0_ext[:, 3 * KT0 * 128:4 * KT0 * 128])
        # qA (sync)
        nc.sync.dma_start(out=x0s[2][:, :], in_=xr_ext[:, 2 * KH:3 * KH])
        nc.sync.dma_start(out=w0b[0][:, :], in_=w0_ext[:, 0:KT0 * 128])
        w0c[2] = wtile(w0_ext, W0C, 8, 2, "w0", nc.sync)
        nc.sync.dma_start(out=xt[1][:, 0:XC // 2], in_=xr_ext[:, XC:XC + XC // 2])
        w0c[4] = wtile(w0_ext, W0C, 8, 4, "w0", nc.sync)
        w0c[6] = wtile(w0_ext, W0C, 8, 6, "w0", nc.sync)
        w1c[0] = wtile(w1_ext, W1C, 4, 0, "w1", nc.sync)
        w1c[2] = wtile(w1_ext, W1C, 4, 2, "w1", nc.sync)
        w2c[0] = wtile(w2_ext, W2C, 4, 0, "w2", nc.sync)
        w2c[2] = wtile(w2_ext, W2C, 4, 2, "w2", nc.sync)
        wtc = wtile(wt_ext, OT2 * 8, 1, 0, "wt", nc.sync)
        t = wp.tile([128, XC], bf16, tag="x2", name="x2")
        nc.sync.dma_start(out=t[:, :], in_=xr_ext[:, 2 * XC:3 * XC])
        xt[2] = t
        # qB (gpsimd) -- critical tiles first, aux strictly after
        nc.gpsimd.dma_start(out=x0s[1][:, :], in_=xr_ext[:, KH:2 * KH])
        nc.gpsimd.dma_start(out=x0s[3][:, :], in_=xr_ext[:, 3 * KH:4 * KH])
        nc.gpsimd.dma_start(out=w0b[2][:, :],
                            in_=w0_ext[:, 2 * KT0 * 128:3 * KT0 * 128])
        b0p = aux(b0_ext, [128, OT0], f32, "b0p", nc.gpsimd)
        nc.gpsimd.dma_start(out=xt[1][:, XC // 2:XC],
                            in_=xr_ext[:, XC + XC // 2:2 * XC])
        w0c[3] = wtile(w0_ext, W0C, 8, 3, "w0", nc.gpsimd)
        w0c[5] = wtile(w0_ext, W0C, 8, 5, "w0", nc.gpsimd)
        w0c[7] = wtile(w0_ext, W0C, 8, 7, "w0", nc.gpsimd)
        b1p = aux(b1_ext, [128, OT1], f32, "b1p", nc.gpsimd)
        w1c[1] = wtile(w1_ext, W1C, 4, 1, "w1", nc.gpsimd)
        w1c[3] = wtile(w1_ext, W1C, 4, 3, "w1", nc.gpsimd)
        b2p = aux(b2_ext, [128, OT2], f32, "b2p", nc.gpsimd)
        w2c[1] = wtile(w2_ext, W2C, 4, 1, "w2", nc.gpsimd)
        w2c[3] = wtile(w2_ext, W2C, 4, 3, "w2", nc.gpsimd)
        bt1c = aux(bt1_ext, [40, 1], f32, "bt1c", nc.gpsimd)
        m2c = aux(m2_ext, [40, 2], bf16, "m2c", nc.gpsimd)
        t = wp.tile([128, XC], bf16, tag="x3", name="x3")
        nc.gpsimd.dma_start(out=t[:, :], in_=xr_ext[:, 3 * XC:4 * XC])
        xt[3] = t

        relu = mybir.ActivationFunctionType.Relu

        def w0sl(o, k):
            if o < 4:
                return w0b[o][:, k * 128:(k + 1) * 128]
            return w0c[o // OB0][:, (o % OB0) * KT0 * 128 + k * 128:
                                 (o % OB0) * KT0 * 128 + (k + 1) * 128]

        def w1sl(o, k):
            return w1c[o // OB1][:, (o % OB1) * KT1 * 128 + k * 128:
                                 (o % OB1) * KT1 * 128 + (k + 1) * 128]

        def w2sl(o, k):
            return w2c[o // OB2][:, (o % OB2) * KT2 * 128 + k * 128:
                                 (o % OB2) * KT2 * 128 + (k + 1) * 128]

        def layer(rhs0, rhs1, kt, ot, wsl, bias_tile, out0, out1,
                  tw=None, o_start=0):
            """One dense layer on the superchunk's chunk pair.

            rhs0/rhs1: fn(k) -> [128, CH] AP.  Each weight tile feeds two
            matmuls.  tw=[16, CH] psum interleaves the tower first-layer
            chains lagged one o-tile behind the backbone.
            """
            for o in range(o_start, ot):
                ps0 = pp_mm.tile([128, CH], f32, tag="mm")
                ps1 = pp_mm.tile([128, CH], f32, tag="mm")
                for k in range(kt):
                    lhsT = wsl(o, k)
                    nc.tensor.matmul(out=ps0[:, :], lhsT=lhsT, rhs=rhs0(k),
                                     start=(k == 0), stop=(k == kt - 1))
                    nc.tensor.matmul(out=ps1[:, :], lhsT=lhsT, rhs=rhs1(k),
                                     start=(k == 0), stop=(k == kt - 1))
                if tw is not None and o > 0:
                    nc.tensor.matmul(out=tw[0:8, :],
                                     lhsT=wtc[:, (o - 1) * 8:o * 8],
                                     rhs=out0[o - 1][:, :],
                                     start=(o == 1), stop=False,
                                     tile_position=(0, 0))
                    nc.tensor.matmul(out=tw[32:40, :],
                                     lhsT=wtc[:, (o - 1) * 8:o * 8],
                                     rhs=out1[o - 1][:, :],
                                     start=(o == 1), stop=False,
                                     tile_position=(0, 32))
                nc.scalar.activation(out=out0[o][:, :], in_=ps0[:, :],
                                     func=relu, bias=bias_tile[:, o:o + 1],
                                     scale=1.0)
                nc.vector.tensor_scalar(out=out1[o][:, :], in0=ps1[:, :],
                                        scalar1=bias_tile[:, o:o + 1],
                                        scalar2=0.0,
                                        op0=mybir.AluOpType.add,
                                        op1=mybir.AluOpType.max)
            if tw is not None:
                o = ot - 1
                nc.tensor.matmul(out=tw[0:8, :], lhsT=wtc[:, o * 8:(o + 1) * 8],
                                 rhs=out0[o][:, :], start=False, stop=True,
                                 tile_position=(0, 0))
                nc.tensor.matmul(out=tw[32:40, :],
                                 lhsT=wtc[:, o * 8:(o + 1) * 8],
                                 rhs=out1[o][:, :], start=False, stop=True,
                                 tile_position=(0, 32))

        def tower_tail(s, ps_tw):
            t1s = sp.tile([40, CH], bf16, tag="t1s")
            nc.vector.tensor_scalar(out=t1s[:, :], in0=ps_tw[:, :],
                                    scalar1=bt1c[:, :], scalar2=0.0,
                                    op0=mybir.AluOpType.add,
                                    op1=mybir.AluOpType.max)
            ps_l = pp_o2.tile([2, CH], f32, tag="twl")
            nc.tensor.matmul(out=ps_l[:, :], lhsT=m2c[:, :], rhs=t1s[:, :],
                             start=True, stop=True)
            outc = sp.tile([2, CH], f32, tag="oc")
            nc.vector.tensor_copy(out=outc[:, :], in_=ps_l[:, :])
            nc.sync.dma_start(out=out_ext[:, s * CH:(s + 1) * CH],
                              in_=outc[:, :])

        for s in range(NSUP):
            c0 = 2 * s
            if s == 0:
                rx0 = lambda k: x0s[k // 2][:, (k % 2) * CH:(k % 2 + 1) * CH]
            else:
                rx0 = lambda k, _t=xt[c0]: _t[:, k * CH:(k + 1) * CH]
            rx1 = lambda k, _t=xt[c0 + 1]: _t[:, k * CH:(k + 1) * CH]
            h1a = [hp.tile([128, CH], bf16, name=f"h1a_{o}", tag=f"h1a_{o}")
                   for o in range(OT0)]
            h1b = [hp.tile([128, CH], bf16, name=f"h1b_{o}", tag=f"h1b_{o}")
                   for o in range(OT0)]
            if s == 0:
                # de-paired o0..o5: chunk-a chains run on x0 alone while x1
                # is still in flight, then chunk-b catches up
                for o in range(6):
                    ps = pp_mm.tile([128, CH], f32, tag="mm")
                    for k in range(KT0):
                        nc.tensor.matmul(out=ps[:, :], lhsT=w0sl(o, k),
                                         rhs=rx0(k), start=(k == 0),
                                         stop=(k == KT0 - 1))
                    nc.scalar.activation(out=h1a[o][:, :], in_=ps[:, :],
                                         func=relu, bias=b0p[:, o:o + 1],
                                         scale=1.0)
                for o in range(6):
                    ps = pp_mm.tile([128, CH], f32, tag="mm")
                    for k in range(KT0):
                        nc.tensor.matmul(out=ps[:, :], lhsT=w0sl(o, k),
                                         rhs=rx1(k), start=(k == 0),
                                         stop=(k == KT0 - 1))
                    nc.vector.tensor_scalar(out=h1b[o][:, :], in0=ps[:, :],
                                            scalar1=b0p[:, o:o + 1],
                                            scalar2=0.0,
                                            op0=mybir.AluOpType.add,
                                            op1=mybir.AluOpType.max)
                layer(rx0, rx1, KT0, OT0, w0sl, b0p, h1a, h1b, o_start=6)
            else:
                layer(rx0, rx1, KT0, OT0, w0sl, b0p, h1a, h1b)
            rh1a = lambda k: h1a[k][:, :]
            rh1b = lambda k: h1b[k][:, :]
            h2a = [hp.tile([128, CH], bf16, name=f"h2a_{o}", tag=f"h2a_{o}")
                   for o in range(OT1)]
            h2b = [hp.tile([128, CH], bf16, name=f"h2b_{o}", tag=f"h2b_{o}")
                   for o in range(OT1)]
            layer(rh1a, rh1b, KT1, OT1, w1sl, b1p, h2a, h2b)
            rh2a = lambda k: h2a[k][:, :]
            rh2b = lambda k: h2b[k][:, :]
            h3a = [hp.tile([128, CH], bf16, name=f"h3a_{o}", tag=f"h3a_{o}")
                   for o in range(OT2)]
            h3b = [hp.tile([128, CH], bf16, name=f"h3b_{o}", tag=f"h3b_{o}")
                   for o in range(OT2)]
            ps_tw = pp_tw.tile([40, CH], f32, tag="tw")
            layer(rh2a, rh2b, KT2, OT2, w2sl, b2p, h3a, h3b, tw=ps_tw)
            tower_tail(s, ps_tw)

    nc.compile()
    return nc


def get_nc():
    global _CACHED_NC
    if _CACHED_NC is None:
        _CACHED_NC = _build()
    return _CACHED_NC


# ---------------- host-side math (exact fp32 mirror of the reference) -------

def _softplus(x):
    return np.logaddexp(0.0, x)


def _ln(x, g, b):
    m = x.mean(-1, keepdims=True)
    v = ((x - m) ** 2).mean(-1, keepdims=True)
    return g * (x - m) / np.sqrt(v + EPS_LN) + b


def _softmax(x):
    e = np.exp(x - x.max(-1, keepdims=True))
    return e / e.sum(-1, keepdims=True)


def _topk_sparse(p, k):
    idx = np.argsort(-p, axis=-1, kind="stable")[..., :k]
    mask = np.zeros_like(p)
    np.put_along_axis(mask, idx, 1.0, axis=-1)
    s = p * mask
    return s / np.maximum(s.sum(-1, keepdims=True), EPS)


def _routing_tables(dom_emb, layer_pos, gate_logits, Wi1, bi1, gi, bti, Wi2,
                    bi2, Wr1, br1, gr, btr, Wr2, br2):
    gate = _softplus(gate_logits.astype(np.float32))
    Rg = np.eye(D, dtype=np.float32) * gate
    Rg = Rg / np.maximum(Rg.sum(1, keepdims=True), EPS)
    hd = Rg @ dom_emb.astype(np.float32)                      # [D, 64]
    ri = np.concatenate([
        np.broadcast_to(hd[:, None, :], (D, L, hd.shape[-1])),
        np.broadcast_to(layer_pos[None].astype(np.float32), (D, L, layer_pos.shape[-1])),
    ], axis=-1)                                               # [D, L, 96]
    hi = np.maximum(_ln(ri @ Wi1.T + bi1, gi, bti), 0.0)
    scores = (hi @ Wi2.T + bi2)[..., 0]
    scores = scores - scores.max(-1, keepdims=True)
    phi = _softmax(scores)
    zeta = _topk_sparse(phi, min(2, L))                       # [D, L]
    hr = np.maximum(_ln(ri @ Wr1.T + br1, gr, btr), 0.0)
    alpha = _topk_sparse(_softmax(hr @ Wr2.T + br2), 2)       # [D, L, E]
    return zeta.astype(np.float32), alpha.astype(np.float32)


def _pack_w(W, kt, ot):
    """[out, in] -> [128, ot*kt*128] with (p, o, k, cc) layout (lhsT blocks)."""
    Wr = np.asarray(W, np.float32).reshape(ot, 128, kt, 128)   # [o, cc, k, p]
    return np.ascontiguousarray(
        Wr.transpose(3, 0, 2, 1).reshape(128, ot * kt * 128)).astype(BF16)


def kernel(field_idx, domain_id, emb_table, W0, b0, W1, b1, W2, b2,
           A0, Bm0, A1, Bm1, A2, Bm2, dom_emb, layer_pos, gate_logits,
           Wi1, bi1, gi, bti, Wi2, bi2, Wr1, br1, gr, btr, Wr2, br2,
           Wt1, bt1, Wt2, bt2):
    field_idx = np.asarray(field_idx)
    domain_id = np.asarray(domain_id).astype(np.int64)
    emb = np.asarray(emb_table, dtype=np.float32)
    W_list = [np.asarray(W, np.float32) for W in (W0, W1, W2)]
    b_list = [np.asarray(b, np.float32) for b in (b0, b1, b2)]
    A_list = [np.asarray(A, np.float32) for A in (A0, A1, A2)]
    Bm_list = [np.asarray(Bm, np.float32) for Bm in (Bm0, Bm1, Bm2)]
    Wt1 = np.asarray(Wt1, np.float32)
    bt1 = np.asarray(bt1, np.float32)
    Wt2 = np.asarray(Wt2, np.float32)
    bt2 = np.asarray(bt2, np.float32)

    zeta, alpha = _routing_tables(
        np.asarray(dom_emb), np.asarray(layer_pos), np.asarray(gate_logits),
        np.asarray(Wi1), np.asarray(bi1), np.asarray(gi), np.asarray(bti),
        np.asarray(Wi2), np.asarray(bi2), np.asarray(Wr1), np.asarray(br1),
        np.asarray(gr), np.asarray(btr), np.asarray(Wr2), np.asarray(br2))

    # fold LoRA delta into per-domain dense weights (exact fp32)
    kts = [KT0, KT1, KT2]
    ots = [OT0, OT1, OT2]
    deltas = []     # deltas[l][d] = zeta*scale * sum_e alpha Bm_e A_e
    for l in range(3):
        Aa, Bb = A_list[l], Bm_list[l]
        dl = []
        for d in range(D):
            w = alpha[d, l, :] * zeta[d, l] * SCALING          # [E]
            act = np.nonzero(w)[0]
            acc = np.zeros((Bb.shape[1], Aa.shape[2]), np.float32)
            for e in act:
                acc += w[e] * (Bb[e] @ Aa[e])
            dl.append(acc)
        deltas.append(dl)

    shared = {
        "b0p": np.ascontiguousarray(b_list[0].reshape(OT0, 128).T),
        "b1p": np.ascontiguousarray(b_list[1].reshape(OT1, 128).T),
        "b2p": np.ascontiguousarray(b_list[2].reshape(OT2, 128).T),
    }

    # host embedding lookup -> feature-major bf16 [IN, B]
    x = emb[field_idx.astype(np.int64)].reshape(B, IN)
    xT = x.T.astype(BF16)

    # assign each core one domain; first BL samples of that domain on
    # device, remainder ("spill") on host in fp32
    order = []            # per core: sample indices (len <= BL)
    spill_idx = []
    for d in range(D):
        idx = np.nonzero(domain_id == d)[0]
        order.append(idx[:BL])
        if idx.shape[0] > BL:
            spill_idx.append(idx[BL:])
    spill_idx = (np.concatenate(spill_idx) if spill_idx
                 else np.empty((0,), np.int64))

    in_maps = []
    for d in range(NCORES):
        idx = order[d]
        xc = np.zeros((IN, BL), dtype=BF16)
        xc[:, :idx.shape[0]] = xT[:, idx]
        xr = np.ascontiguousarray(
            xc.reshape(KT0, 128, NCHUNK, CH).transpose(1, 2, 0, 3)
            .reshape(128, NCHUNK * KT0 * CH))
        m = dict(shared)
        m["xr"] = xr
        for l, nm in enumerate(("w0c", "w1c", "w2c")):
            m[nm] = _pack_w(W_list[l] + deltas[l][d], kts[l], ots[l])
        # tower: Wt1[d]: [8, 512] -> lhsT [512, 8] as (p, k, cc)
        wtt = Wt1[d].T.reshape(OT2, 128, 8)                    # [k, p, cc]
        m["wtc"] = np.ascontiguousarray(
            wtt.transpose(1, 0, 2).reshape(128, OT2 * 8)).astype(BF16)
        btc = np.zeros((40, 1), np.float32)
        btc[0:8, 0] = bt1[d]
        btc[32:40, 0] = bt1[d]
        m["bt1c"] = btc
        m2 = np.zeros((40, 2), np.float32)
        m2[0:8, 0] = Wt2[d, 0, :]
        m2[32:40, 1] = Wt2[d, 0, :]
        m["m2c"] = m2.astype(BF16)
        in_maps.append(m)

    nc = get_nc()
    res = bass_utils.run_bass_kernel_spmd(nc, in_maps, core_ids=list(range(NCORES)))
    out = np.empty(B, dtype=np.float32)
    for d in range(NCORES):
        lg = np.asarray(res.results[d]["out"], np.float32)     # [2, NSUP*CH]
        idx = order[d]
        j = np.arange(idx.shape[0])
        s, r = j // (2 * CH), j % (2 * CH)
        half, jj = r // CH, r % CH
        out[idx] = lg[half, s * CH + jj] + bt2[d, 0]

    # spill samples: exact fp32 on host
    if spill_idx.shape[0]:
        hs = x[spill_idx]                                      # [n, IN]
        ds = domain_id[spill_idx]
        for l in range(3):
            base = hs @ W_list[l].T + b_list[l]
            dlt = np.stack([deltas[l][d] for d in range(D)], 0)  # [D, out, in]
            lora = np.einsum('ni,noi->no', hs, dlt[ds])
            hs = np.maximum(base + lora, 0.0)
        t1 = np.maximum(np.einsum('nf,nof->no', hs, Wt1[ds]) + bt1[ds], 0.0)
        out[spill_idx] = (np.einsum('no,nio->ni', t1, Wt2[ds])[:, 0]
                          + bt2[ds, 0])
    return out
